# revision 20
# baseline (speedup 1.0000x reference)
"""Joint bilateral filter (3x3, reflect pad) on 8 trn2 cores.

Sharding: 1024 output rows (2 batches x 512 H) split as 8 x 128 rows.
Host pre-pads H and W with reflect (radius 1), so each core gets a
halo-inclusive channel-major shard and computes its [128, C, 512]
output slab with no boundary handling on device.

Device layout: partition p = output row p of the shard. Free dim is
channel-major [C, W] so the per-pixel bilateral weight (one per W pos)
broadcasts across channels via a stride-0 AP, and dx shifts are free-dim
offsets. dy shifts are handled by loading 3 row-shifted copies of the
inputs (dy = 0,1,2 -> padded rows [dy, dy+128)).

Key structure, per column chunk:
- The 3x3 spatial kernel is folded into PE's weight matrices: the host
  sends three scaled identities w1*I (w1 = exp(-s/2), s = squared tap
  distance in {0,1,2}) and each tap's PSUM accumulate uses the identity
  matching its spatial weight. The guide weights wk are therefore raw
  exp(-8*||guide diff||^2): one batched square, two batched channel-sum
  adds, and ONE batched exp with shared scale for all 7 fields.
- den = 1 + sum_k w1_k*wk_k is ALSO accumulated on PE: eight 128-free
  matmuls (w1*I @ wk_slot) into a 1-bank PSUM tile, +1 via an Act
  Copy-with-bias into SBUF. Pool's only remaining weight work is the
  two batched channel-sum adds.
- DVE:  guide-difference subs (one op per dy group covering its three
  dx shifts via a hand-built overlapping 3-dim AP), ~6.4 of the 8
  non-center tap products in bf16 2x packed mode, reciprocal of den,
  and the final num*(1/den).
- Pool: the k=1 tap product and most of the k=7 product (the column
  split balances DVE vs Pool), plus the channel sums. weights(ci+1)
  is emitted before the Pool products of mac(ci), so the channel sums
  sit early in Pool's queue and the next chunk's exp is never gated
  on Pool's slow products.
- PE:   num = center + sum of 8 tap products via identity-weight
  matmuls into two half-width PSUM tiles (the adds cost the
  otherwise-idle tensor engine ~1.1us per tap instead of 1.5us of
  DVE, and fp32 PSUM accumulation improves accuracy). A narrow filler
  matmul between taps keeps the PE's DVFS ramped through prod waits
  (2.4GHz needs 3us of gap-free execution; any bubble resets it).
- tap symmetry: w5[p,x] = w3[p,x+1] exactly, so tap 5's weight is an
  offset view into k=3's slot of the batched weight tile.
- chunk widths are [112, 128, 128, 112, 32]: a short first chunk
  (split further into two column halves all the way through
  weights+mac) shortens the serial startup of the weight pipeline,
  and a tiny last chunk shrinks the drain tail. All SBUF tiles are
  allocated at the 128-col maximum (same-tag tiles must keep one
  shape); narrower chunks just use a prefix, and their DMAs write a
  sliced prefix of the tile.
- src arrives as per-chunk row-shifted slabs, one contiguous run per
  partition; the guide arrives as ONE merged per-chunk DMA and the
  identities as one merged DMA - HWDGE grants cost 625ns each, so
  fewer DMA dispatches start the pipeline sooner. The output DRAM is
  per-chunk so each out-DMA descriptor is one contiguous multi-KB run
  per partition (runs under 512B pay 2x in the DMA engines).
- each chunk's finalize (reciprocal etc) is emitted one chunk late so
  the in-order DVE stream never stalls on den/num completion.
- center tap weight is exactly 1: PE accumulates src directly, +1 for
  den via the Act bias.
"""

import sys

sys.path.insert(0, "/opt/trn_rl_repo")

import math

import ml_dtypes
import numpy as np

BF16 = ml_dtypes.bfloat16

B, H, W = 2, 512, 512
CS, CI = 21, 3
N_CORES = 8
ROWS = (B * H) // N_CORES  # 128 output rows per core
WP = W + 2  # padded width

CHUNKS = [112, 128, 128, 112, 32]
OFFS = [0, 112, 240, 368, 480]
N_CHUNKS = len(CHUNKS)
MAXW = 128
SLABW = MAXW + 2  # slab tile width
IMTW = MAXW + 4  # guide tile width

SIGMA_BILATERAL = 0.25
INV2SIG2 = 1.0 / (2.0 * SIGMA_BILATERAL**2)  # 8.0

# weight-field slots in the batched D/WR/WK tiles:
#   slot 0..2 = taps (dy=0, dx=0..2) = k0,k1,k2
#   slot 3..5 = taps (dy=2, dx=0..2) = k6,k7,k8
#   slot 6    = tap  (dy=1, dx=0)    = k3   (k5 = slot 6, one col right)
SLOT_OF = {0: 0, 1: 1, 2: 2, 6: 3, 7: 4, 8: 5, 3: 6}
# spatial-kernel index per tap: (dx-1)^2 + (dy-1)^2 in {0,1,2}
SIDX = {k: ((k % 3) - 1) ** 2 + ((k // 3) - 1) ** 2 for k in range(9)}

_CACHE = {}


def _ap_with(ap, dims):
    """Copy `ap` and replace its free dims (keeps partition dim + offset)."""
    import bass_rust

    c = ap.copy()
    part = list(c.ap)[0]
    c.ap = bass_rust.VecI64Pair([list(part)] + [list(d) for d in dims])
    return c


def _build():
    from concourse.bacc import Bacc
    from concourse.tile import TileContext
    import concourse.mybir as mybir

    fp32 = mybir.dt.float32
    bf16 = mybir.dt.bfloat16
    Alu = mybir.AluOpType
    Act = mybir.ActivationFunctionType

    nc = Bacc("TRN2", target_bir_lowering=False, debug=False, num_devices=N_CORES)
    se_d, im_d, out_d = [], [], []
    for ci, w in enumerate(CHUNKS):
        se_d.append(
            nc.dram_tensor(f"src{ci}", [ROWS + 2, CS, w + 2], bf16, kind="ExternalInput")
        )
        im_d.append(
            nc.dram_tensor(f"img{ci}", [ROWS, 3, CI, w + 4], bf16, kind="ExternalInput")
        )
        ow = MAXW if ci < N_CHUNKS - 1 else w
        out_d.append(nc.dram_tensor(f"out{ci}", [ROWS, CS, ow], bf16, kind="ExternalOutput"))
    # three spatially-scaled identities: exp(-s/2) * I for s = 0, 1, 2
    id_d = nc.dram_tensor("ident", [3, ROWS, ROWS], bf16, kind="ExternalInput")

    # num PSUM half-windows: channel ranges of <= 8 channels
    # (8*64 = 512 fp32, one 2KB bank per window)
    WIN2 = [(c0, min(CS, c0 + 8)) for c0 in range(0, CS, 8)]

    with TileContext(nc) as tc:
        with tc.tile_pool(name="p", bufs=1) as pool, tc.tile_pool(
            name="ps", bufs=1, space="PSUM"
        ) as ppool:
            # ---- input DMAs, interleaved for a fast chunk-0 start -------
            imt = [None] * N_CHUNKS
            s_e = [[None] * 3 for _ in range(N_CHUNKS)]

            def issue_im(ci):
                w = CHUNKS[ci]
                it = pool.tile([ROWS, 3, CI, IMTW], bf16, tag="imt", bufs=2)
                src = im_d[ci][:]
                if w + 4 == IMTW:
                    nc.sync.dma_start(
                        it[:].rearrange("p a c w -> p (a c w)"),
                        src.rearrange("p a c w -> p (a c w)"),
                    )
                else:
                    nc.sync.dma_start(
                        _ap_with(it[:], [[IMTW, 9], [1, w + 4]]),
                        _ap_with(src, [[w + 4, 9], [1, w + 4]]),
                    )
                imt[ci] = it

            def issue_slab(ci, dys):
                w = CHUNKS[ci]
                for dy in dys:
                    se = pool.tile([ROWS, CS, SLABW], bf16, tag=f"se{dy}", bufs=2)
                    src = se_d[ci][dy : dy + ROWS]
                    if w + 2 == SLABW:
                        nc.sync.dma_start(
                            se[:].rearrange("p c w -> p (c w)"),
                            src.rearrange("p c w -> p (c w)"),
                        )
                    else:
                        nc.sync.dma_start(se[:, :, 0 : w + 2], src)
                    s_e[ci][dy] = se

            issue_im(0)
            issue_slab(0, (0,))
            ident = pool.tile([ROWS, 3, ROWS], bf16, tag="ident")
            nc.sync.dma_start(ident[:], id_d[:].transpose([1, 0, 2]))
            issue_slab(0, (1, 2))
            for ci in range(1, N_CHUNKS):
                issue_im(ci)
                issue_slab(ci, (0, 1, 2))

            def idw(s):
                return ident[:, s]

            # Each consuming engine observes every slab DMA once (tiny
            # absorber ops) so real consumers don't pile up sync waits.
            dummV = pool.tile([1, 1, 1], bf16, tag="dummV")
            dummP = pool.tile([1, 1, 1], bf16, tag="dummP")

            def absorb_src_slab(ci):
                for t in s_e[ci]:
                    nc.vector.tensor_scalar(
                        dummV[:], t[0:1, 0:1, 0:1], 0.0, None, Alu.add
                    )

            # --- software-pipelined chunk loop ---------------------------
            wk_all = [None] * N_CHUNKS
            dens_all = [None] * N_CHUNKS
            pp1_all = [None] * N_CHUNKS
            pp7_all = [None] * N_CHUNKS

            def segs_of(ci):
                w = CHUNKS[ci]
                return [(0, w // 2), (w // 2, w)] if ci == 0 else [(0, w)]

            def psplit(ci):
                w = CHUNKS[ci]
                return (w * 5) // 8 if w >= 64 else 0

            def wk_view(ci, k, a, b):
                wk = wk_all[ci]
                s = SLOT_OF[3 if k == 5 else k]
                off = (1 if k == 5 else 0) + a
                return (
                    wk[:, s : s + 1, off : off + (b - a)]
                    .broadcast_to([ROWS, CS, b - a])
                )

            def weights(ci):
                w = CHUNKS[ci]
                it = imt[ci]
                d = pool.tile([ROWS, 7, CI, SLABW], bf16, tag="d", bufs=2)
                d2 = pool.tile([ROWS, 7, CI, SLABW], fp32, tag="d2", bufs=2)
                wr = pool.tile([ROWS, 7, SLABW], fp32, tag="wr", bufs=2)
                wk = pool.tile([ROWS, 7, SLABW], bf16, tag="wk", bufs=2)
                dcw = CI * SLABW
                fsegs = (
                    [(0, w // 2 + 2), (w // 2 + 2, w + 2)] if ci == 0 else [(0, w + 2)]
                )
                for a, b in fsegs:
                    n = b - a
                    # the three dx-shifted fields of each row-shifted dy
                    # group in ONE sub via an overlapping (dx, c, x) AP
                    # (engine APs allow at most 3 free dims: one op per dy)
                    for g, dy in enumerate((0, 2)):
                        mn3 = _ap_with(it[:, dy, :, a:], [[1, 3], [IMTW, 3], [1, n]])
                        ct3 = _ap_with(
                            it[:, 1, :, a + 1 :], [[0, 3], [IMTW, 3], [1, n]]
                        )
                        o3 = _ap_with(d[:, 3 * g, :, a:], [[dcw, 3], [SLABW, 3], [1, n]])
                        nc.vector.tensor_tensor(o3, mn3, ct3, Alu.subtract)
                    # k=3 field (dy=1, dx=0)
                    nc.vector.tensor_tensor(
                        d[:, 6, :, a:b], it[:, 1, :, a:b],
                        it[:, 1, :, a + 1 : b + 1], Alu.subtract,
                    )
                    # one batched square of all 7 fields
                    nc.scalar.square(d2[:, :, :, a:b], d[:, :, :, a:b])
                    # channel sums on Pool, batched across the 7 fields
                    nc.gpsimd.tensor_tensor(
                        wr[:, :, a:b], d2[:, :, 0, a:b], d2[:, :, 1, a:b], Alu.add
                    )
                    nc.gpsimd.tensor_tensor(
                        wr[:, :, a:b], d2[:, :, 2, a:b], wr[:, :, a:b], Alu.add
                    )
                    # one batched exp (raw guide weights; spatial factors
                    # live in PE's scaled identities)
                    nc.scalar.activation(
                        wk[:, :, a:b], wr[:, :, a:b], Act.Exp,
                        bias=0.0, scale=-INV2SIG2,
                    )
                wk_all[ci] = wk

            def pp1(ci):
                # Pool tap products; emitted after weights(ci+1) so the
                # channel sums clear Pool's queue first
                nc.gpsimd.tensor_scalar(
                    dummP[:], s_e[ci][0][0:1, 0:1, 0:1], 0.0, None, Alu.add
                )
                nc.gpsimd.tensor_scalar(
                    dummP[:], s_e[ci][2][0:1, 0:1, 0:1], 0.0, None, Alu.add
                )
                ps = psplit(ci)
                pt = pool.tile([ROWS, CS, MAXW], bf16, tag="prod1")
                pt7 = pool.tile([ROWS, CS, MAXW], bf16, tag="prod7")
                for a, b in segs_of(ci):
                    nc.gpsimd.tensor_tensor(
                        pt[:, :, a:b], s_e[ci][0][:, :, 1 + a : 1 + b],
                        wk_view(ci, 1, a, b), Alu.mult,
                    )
                    # Pool's share of the k=7 product
                    pa, pb = a, min(b, ps)
                    if pa < pb:
                        nc.gpsimd.tensor_tensor(
                            pt7[:, :, pa:pb], s_e[ci][2][:, :, 1 + pa : 1 + pb],
                            wk_view(ci, 7, pa, pb), Alu.mult,
                        )
                pp1_all[ci] = pt
                pp7_all[ci] = pt7

            def mac(ci):
                w = CHUNKS[ci]
                ps = psplit(ci)
                wk = wk_all[ci]
                absorb_src_slab(ci)
                nc.vector.tensor_scalar(
                    dummV[:], wk[0:1, 0:1, 0:1], 0.0, None, Alu.add
                )
                prods = {1: pp1_all[ci], 7: pp7_all[ci]}
                # DVE products, ordered by slab arrival (dy 0, 1, 2);
                # k=7's tail columns complement Pool's share
                for k in (0, 2, 3, 5, 6, 8):
                    dy, dx = k // 3, k % 3
                    pt = pool.tile([ROWS, CS, MAXW], bf16, tag=f"prod{k}")
                    for a, b in segs_of(ci):
                        nc.vector.tensor_tensor(
                            pt[:, :, a:b], s_e[ci][dy][:, :, dx + a : dx + b],
                            wk_view(ci, k, a, b), Alu.mult,
                        )
                    prods[k] = pt
                for a, b in segs_of(ci):
                    da, db = max(a, ps), b
                    if da < db:
                        nc.vector.tensor_tensor(
                            prods[7][:, :, da:db],
                            s_e[ci][2][:, :, 1 + da : 1 + db],
                            wk_view(ci, 7, da, db), Alu.mult,
                        )

                # PE part 1: den = sum_k w1_k * wk_k in a 1-bank PSUM tile
                denp = ppool.tile([ROWS, MAXW], fp32, tag="denp")
                dks = (0, 1, 2, 3, 6, 7, 8, 5)
                for a, b in segs_of(ci):
                    for i, k in enumerate(dks):
                        s = SLOT_OF[3 if k == 5 else k]
                        off = (1 if k == 5 else 0) + a
                        nc.tensor.matmul(
                            denp[:, a:b], idw(SIDX[k]),
                            wk[:, s, off : off + (b - a)],
                            start=(i == 0), stop=(i == len(dks) - 1),
                        )
                # den -> SBUF with the center tap's +1 folded into the bias
                dens = pool.tile([ROWS, MAXW], fp32, tag=f"dens{ci}")
                nc.scalar.activation(dens[:, 0:w], denp[:, 0:w], Act.Copy, bias=1.0)
                dens_all[ci] = dens

                # PE part 2: num = center + sum of tap products, in two
                # half-width PSUM tiles; narrow fillers keep DVFS ramped
                half = w // 2
                numps = []
                for h in range(2):
                    np_h = ppool.tile([ROWS, CS, MAXW // 2], fp32, tag=f"nump{h}")
                    numps.append(np_h)
                fill = ppool.tile([ROWS, MAXW], fp32, tag="fill")

                def filler(n):
                    for _ in range(n):
                        nc.tensor.matmul(
                            fill[:], idw(0), s_e[ci][0][:, 0:1, 0:MAXW],
                            start=True, stop=True,
                        )

                acc = [(0, s_e[ci][1][:, :, 1 : 1 + w])]
                acc += [(SIDX[k], prods[k][:, :, 0:w]) for k in (0, 2, 3, 5, 6, 8, 7, 1)]
                n_acc = len(acc)
                for t, (sx, ap) in enumerate(acc):
                    for h in range(2):
                        for a, b in WIN2:
                            nc.tensor.matmul(
                                numps[h][:, a:b, 0:half],
                                idw(sx),
                                ap[:, a:b, h * half : (h + 1) * half],
                                start=(t == 0), stop=(t == n_acc - 1),
                            )
                    if t < n_acc - 1:
                        filler(1)

                # Act: PSUM -> SBUF (bf16) per-half copies of num
                numb = pool.tile([ROWS, CS, MAXW], bf16, tag="numb", bufs=2)
                for h in range(2):
                    nc.scalar.copy(
                        numb[:, :, h * half : (h + 1) * half],
                        numps[h][:, :, 0:half],
                    )

                def finalize():
                    dens = dens_all[ci]
                    rd = pool.tile([ROWS, MAXW], fp32, tag=f"rd{ci}")
                    nc.vector.reciprocal(rd[:, 0:w], dens[:, 0:w])
                    rdb = pool.tile([ROWS, MAXW], bf16, tag=f"rdb{ci}")
                    nc.vector.tensor_scalar(rdb[:, 0:w], rd[:, 0:w], 0.0, None, Alu.add)
                    last = ci == N_CHUNKS - 1
                    outt = pool.tile(
                        [ROWS, CS, CHUNKS[-1] if last else MAXW],
                        bf16, tag="outtL" if last else "outt",
                        bufs=1 if last else 2,
                    )
                    for a, b in [(0, w)]:
                        rdb_b = rdb[:, a:b].rearrange(
                            "p (x w) -> p x w", x=1
                        ).broadcast_to([ROWS, CS, b - a])
                        nc.vector.tensor_tensor(
                            outt[:, :, a:b], numb[:, :, a:b], rdb_b, Alu.mult
                        )
                        if last or w == MAXW:
                            nc.sync.dma_start(out_d[ci][:], outt[:])
                        else:
                            # pad-width DMA: cols >= w are junk, host slices
                            nc.sync.dma_start(out_d[ci][:], outt[:])

                    return None

                return finalize

            pending_finalize = None
            weights(0)
            for ci in range(N_CHUNKS):
                if ci + 1 < N_CHUNKS:
                    weights(ci + 1)
                pp1(ci)
                fin = mac(ci)
                if pending_finalize is not None:
                    pending_finalize()
                pending_finalize = fin
            pending_finalize()
    nc.compile()
    return nc


def _get_nc():
    if "nc" not in _CACHE:
        _CACHE["nc"] = _build()
    return _CACHE["nc"]


def _shard_inputs(src, im):
    srcp = np.pad(src, ((0, 0), (1, 1), (1, 1), (0, 0)), mode="reflect")
    imp = np.pad(im, ((0, 0), (1, 1), (1, 1), (0, 0)), mode="reflect")
    # channel-major: [B, Hp, C, Wp], bf16; pad junk cols so slab/im slices
    # below stay in range
    srcp = np.transpose(srcp, (0, 1, 3, 2)).astype(BF16)
    srcp = np.pad(srcp, ((0, 0), (0, 0), (0, 0), (0, 2)))
    imp = np.transpose(imp, (0, 1, 3, 2)).astype(BF16)
    imp = np.pad(imp, ((0, 0), (0, 0), (0, 0), (0, OFFS[-1] + CHUNKS[-1] + 4 - WP)))
    ident = np.stack(
        [(np.eye(ROWS) * math.exp(-0.5 * s)).astype(BF16) for s in range(3)]
    )
    in_maps = []
    for core in range(N_CORES):
        b, r0 = core // 4, (core % 4) * ROWS
        sl = srcp[b, r0 : r0 + ROWS + 2]  # [130, 21, 516]
        imc = imp[b, r0 : r0 + ROWS + 2]  # [130, 3, >=516]
        m = {"ident": ident}
        for ci, w in enumerate(CHUNKS):
            off = OFFS[ci]
            m[f"src{ci}"] = np.ascontiguousarray(sl[:, :, off : off + w + 2])
            m[f"img{ci}"] = np.ascontiguousarray(
                np.stack(
                    [imc[dy : dy + ROWS, :, off : off + w + 4] for dy in range(3)],
                    axis=1,
                )
            )
        in_maps.append(m)
    return in_maps


def kernel(src, im, _trace=False, _tmpdir=None):
    from concourse import bass_utils

    src = np.asarray(src, dtype=np.float32)
    im = np.asarray(im, dtype=np.float32)
    nc = _get_nc()
    in_maps = _shard_inputs(src, im)
    res = bass_utils.run_bass_kernel_spmd(
        nc, in_maps, core_ids=list(range(N_CORES)), trace=_trace, tmpdir=_tmpdir
    )
    out = np.empty((B, H, W, CS), dtype=np.float32)
    for core in range(N_CORES):
        b, r0 = core // 4, (core % 4) * ROWS
        r = res.results[core]
        for ci, w in enumerate(CHUNKS):
            off = OFFS[ci]
            o = r[f"out{ci}"][:, :, 0:w]  # [128, 21, w]
            out[b, r0 : r0 + ROWS, off : off + w, :] = np.transpose(
                o, (0, 2, 1)
            ).astype(np.float32)
    _CACHE["last_results"] = res
    return out


# revision 26
# speedup vs baseline: 1.0778x; 1.0778x over previous
"""Joint bilateral filter (3x3, reflect pad) on 8 trn2 cores.

Sharding: 1024 output rows (2 batches x 512 H) split as 8 x 128 rows.
Host pre-pads H and W with reflect (radius 1), so each core gets a
halo-inclusive channel-major shard and computes its [128, C, 512]
output slab with no boundary handling on device.

Device layout: partition p = output row p of the shard. Free dim is
channel-major [C, W] so the per-pixel bilateral weight (one per W pos)
broadcasts across channels via a stride-0 AP, and dx shifts are free-dim
offsets. dy shifts are handled by loading 3 row-shifted copies of the
inputs (dy = 0,1,2 -> padded rows [dy, dy+128)).

Key structure, per column chunk:
- The 3x3 spatial kernel is folded into PE's weight matrices: the host
  sends three scaled identities w1*I (w1 = exp(-s/2), s = squared tap
  distance in {0,1,2}) and each tap's PSUM accumulate uses the identity
  matching its spatial weight. The guide weights wk are therefore raw
  exp(-8*||guide diff||^2): one batched square, two batched channel-sum
  adds, and ONE batched exp with shared scale for all 7 fields.
- den = 1 + sum_k w1_k*wk_k is ALSO accumulated on PE: eight 128-free
  matmuls (w1*I @ wk_slot) into a 1-bank PSUM tile, +1 via an Act
  Copy-with-bias into SBUF. Pool's only remaining weight work is the
  two batched channel-sum adds.
- DVE:  guide-difference subs (one op per dy group covering its three
  dx shifts via a hand-built overlapping 3-dim AP), ~6.4 of the 8
  non-center tap products in bf16 2x packed mode, reciprocal of den,
  and the final num*(1/den).
- Pool: the k=1 tap product and most of the k=7 product (the column
  split balances DVE vs Pool), plus the channel sums. weights(ci+1)
  is emitted before the Pool products of mac(ci), so the channel sums
  sit early in Pool's queue and the next chunk's exp is never gated
  on Pool's slow products.
- PE:   num = center + sum of 8 tap products via identity-weight
  matmuls into two half-width PSUM tiles (the adds cost the
  otherwise-idle tensor engine ~1.1us per tap instead of 1.5us of
  DVE, and fp32 PSUM accumulation improves accuracy). A narrow filler
  matmul between taps keeps the PE's DVFS ramped through prod waits
  (2.4GHz needs 3us of gap-free execution; any bubble resets it).
- tap symmetry: w5[p,x] = w3[p,x+1] exactly, so tap 5's weight is an
  offset view into k=3's slot of the batched weight tile.
- chunk widths are [112, 128, 128, 112, 32]: a short first chunk
  (split further into two column halves all the way through
  weights+mac) shortens the serial startup of the weight pipeline,
  and a tiny last chunk shrinks the drain tail. All SBUF tiles are
  allocated at the 128-col maximum (same-tag tiles must keep one
  shape); narrower chunks just use a prefix, and their DMAs write a
  sliced prefix of the tile.
- src arrives as per-chunk row-shifted slabs, one contiguous run per
  partition; the guide arrives as ONE merged per-chunk DMA and the
  identities as one merged DMA - HWDGE grants cost 625ns each, so
  fewer DMA dispatches start the pipeline sooner. The output DRAM is
  per-chunk so each out-DMA descriptor is one contiguous multi-KB run
  per partition (runs under 512B pay 2x in the DMA engines).
- each chunk's finalize (reciprocal etc) is emitted one chunk late so
  the in-order DVE stream never stalls on den/num completion.
- center tap weight is exactly 1: PE accumulates src directly, +1 for
  den via the Act bias.
"""

import sys

sys.path.insert(0, "/opt/trn_rl_repo")

import math

import ml_dtypes
import numpy as np

BF16 = ml_dtypes.bfloat16

B, H, W = 2, 512, 512
CS, CI = 21, 3
N_CORES = 8
ROWS = (B * H) // N_CORES  # 128 output rows per core
WP = W + 2  # padded width

CHUNKS = [112, 128, 128, 112, 32]
OFFS = [0, 112, 240, 368, 480]
N_CHUNKS = len(CHUNKS)
MAXW = 128
SLABW = MAXW + 2  # slab tile width
IMTW = MAXW + 4  # guide tile width

SIGMA_BILATERAL = 0.25
INV2SIG2 = 1.0 / (2.0 * SIGMA_BILATERAL**2)  # 8.0

# weight-field slots in the batched D/WR/WK tiles:
#   slot 0..2 = taps (dy=0, dx=0..2) = k0,k1,k2
#   slot 3..5 = taps (dy=2, dx=0..2) = k6,k7,k8
#   slot 6    = tap  (dy=1, dx=0)    = k3   (k5 = slot 6, one col right)
SLOT_OF = {0: 0, 1: 1, 2: 2, 6: 3, 7: 4, 8: 5, 3: 6}
# spatial-kernel index per tap: (dx-1)^2 + (dy-1)^2 in {0,1,2}
SIDX = {k: ((k % 3) - 1) ** 2 + ((k // 3) - 1) ** 2 for k in range(9)}

_CACHE = {}


def _ap_with(ap, dims):
    """Copy `ap` and replace its free dims (keeps partition dim + offset)."""
    import bass_rust

    c = ap.copy()
    part = list(c.ap)[0]
    c.ap = bass_rust.VecI64Pair([list(part)] + [list(d) for d in dims])
    return c


def _build():
    from concourse.bacc import Bacc
    from concourse.tile import TileContext
    import concourse.mybir as mybir

    fp32 = mybir.dt.float32
    bf16 = mybir.dt.bfloat16
    Alu = mybir.AluOpType
    Act = mybir.ActivationFunctionType

    nc = Bacc("TRN2", target_bir_lowering=False, debug=False, num_devices=N_CORES)
    se_d, im_d, out_d = [], [], []
    for ci, w in enumerate(CHUNKS):
        # slabs/guides of near-max chunks are padded to the tile width so
        # the load stays one contiguous >=512B run per partition; only the
        # tiny tail chunk uses a sliced (min-time-descriptor) load
        sw = SLABW if w + 2 > SLABW // 2 else w + 2
        se_d.append(
            nc.dram_tensor(f"src{ci}", [ROWS + 2, CS, sw], bf16, kind="ExternalInput")
        )
        im_d.append(
            nc.dram_tensor(f"img{ci}", [ROWS, 3, CI, IMTW], bf16, kind="ExternalInput")
        )
        ow = MAXW if ci < N_CHUNKS - 1 else w
        out_d.append(nc.dram_tensor(f"out{ci}", [ROWS, CS, ow], bf16, kind="ExternalOutput"))
    # three spatially-scaled identities: exp(-s/2) * I for s = 0, 1, 2
    id_d = nc.dram_tensor("ident", [3, ROWS, ROWS], bf16, kind="ExternalInput")

    # num PSUM half-windows: channel ranges of <= 8 channels
    # (8*64 = 512 fp32, one 2KB bank per window)
    WIN2 = [(c0, min(CS, c0 + 8)) for c0 in range(0, CS, 8)]

    with TileContext(nc) as tc:
        with tc.tile_pool(name="p", bufs=1) as pool, tc.tile_pool(
            name="ps", bufs=1, space="PSUM"
        ) as ppool:
            # ---- input DMAs, interleaved for a fast chunk-0 start -------
            imt = [None] * N_CHUNKS
            s_e = [[None] * 3 for _ in range(N_CHUNKS)]

            def issue_im(ci):
                it = pool.tile([ROWS, 3, CI, IMTW], bf16, tag="imt", bufs=2)
                nc.sync.dma_start(
                    it[:].rearrange("p a c w -> p (a c w)"),
                    im_d[ci][:].rearrange("p a c w -> p (a c w)"),
                )
                imt[ci] = it

            def issue_slab(ci, dys):
                w = CHUNKS[ci]
                sw = SLABW if w + 2 > SLABW // 2 else w + 2
                for dy in dys:
                    se = pool.tile([ROWS, CS, SLABW], bf16, tag=f"se{dy}", bufs=2)
                    src = se_d[ci][dy : dy + ROWS]
                    if sw == SLABW:
                        nc.sync.dma_start(
                            se[:].rearrange("p c w -> p (c w)"),
                            src.rearrange("p c w -> p (c w)"),
                        )
                    else:
                        nc.sync.dma_start(se[:, :, 0:sw], src)
                    s_e[ci][dy] = se

            issue_im(0)
            issue_slab(0, (0,))
            ident = pool.tile([ROWS, 3, ROWS], bf16, tag="ident")
            nc.sync.dma_start(ident[:], id_d[:].transpose([1, 0, 2]))
            issue_slab(0, (1, 2))
            for ci in range(1, N_CHUNKS):
                issue_im(ci)
                issue_slab(ci, (0, 1, 2))

            def idw(s):
                return ident[:, s]

            # Each consuming engine observes every slab DMA once (tiny
            # absorber ops) so real consumers don't pile up sync waits.
            dummV = pool.tile([1, 1, 1], bf16, tag="dummV")
            dummP = pool.tile([1, 1, 1], bf16, tag="dummP")

            def absorb_src_slab(ci):
                for t in s_e[ci]:
                    nc.vector.tensor_scalar(
                        dummV[:], t[0:1, 0:1, 0:1], 0.0, None, Alu.add
                    )

            # --- software-pipelined chunk loop ---------------------------
            wk_all = [None] * N_CHUNKS
            dens_all = [None] * N_CHUNKS
            pp1_all = [None] * N_CHUNKS
            pp7_all = [None] * N_CHUNKS

            def segs_of(ci):
                w = CHUNKS[ci]
                return [(0, w // 2), (w // 2, w)] if ci == 0 else [(0, w)]

            def psplit(ci):
                w = CHUNKS[ci]
                return (w * 5) // 8 if w >= 64 else 0

            def wk_view(ci, k, a, b):
                wk = wk_all[ci]
                s = SLOT_OF[3 if k == 5 else k]
                off = (1 if k == 5 else 0) + a
                return (
                    wk[:, s : s + 1, off : off + (b - a)]
                    .broadcast_to([ROWS, CS, b - a])
                )

            def weights(ci):
                w = CHUNKS[ci]
                it = imt[ci]
                d = pool.tile([ROWS, 7, CI, SLABW], bf16, tag="d", bufs=2)
                d2 = pool.tile([ROWS, 7, CI, SLABW], fp32, tag="d2", bufs=2)
                wr = pool.tile([ROWS, 7, SLABW], fp32, tag="wr", bufs=2)
                wk = pool.tile([ROWS, 7, SLABW], bf16, tag="wk", bufs=2)
                dcw = CI * SLABW
                fsegs = (
                    [(0, w // 2 + 2), (w // 2 + 2, w + 2)] if ci == 0 else [(0, w + 2)]
                )
                for a, b in fsegs:
                    n = b - a
                    # the three dx-shifted fields of each row-shifted dy
                    # group in ONE sub via an overlapping (dx, c, x) AP
                    # (engine APs allow at most 3 free dims: one op per dy)
                    for g, dy in enumerate((0, 2)):
                        mn3 = _ap_with(it[:, dy, :, a:], [[1, 3], [IMTW, 3], [1, n]])
                        ct3 = _ap_with(
                            it[:, 1, :, a + 1 :], [[0, 3], [IMTW, 3], [1, n]]
                        )
                        o3 = _ap_with(d[:, 3 * g, :, a:], [[dcw, 3], [SLABW, 3], [1, n]])
                        nc.vector.tensor_tensor(o3, mn3, ct3, Alu.subtract)
                    # k=3 field (dy=1, dx=0)
                    nc.vector.tensor_tensor(
                        d[:, 6, :, a:b], it[:, 1, :, a:b],
                        it[:, 1, :, a + 1 : b + 1], Alu.subtract,
                    )
                    # one batched square of all 7 fields
                    nc.scalar.square(d2[:, :, :, a:b], d[:, :, :, a:b])
                    # channel sums on Pool, batched across the 7 fields
                    nc.gpsimd.tensor_tensor(
                        wr[:, :, a:b], d2[:, :, 0, a:b], d2[:, :, 1, a:b], Alu.add
                    )
                    nc.gpsimd.tensor_tensor(
                        wr[:, :, a:b], d2[:, :, 2, a:b], wr[:, :, a:b], Alu.add
                    )
                    # one batched exp (raw guide weights; spatial factors
                    # live in PE's scaled identities)
                    nc.scalar.activation(
                        wk[:, :, a:b], wr[:, :, a:b], Act.Exp,
                        bias=0.0, scale=-INV2SIG2,
                    )
                wk_all[ci] = wk

            def pp1(ci):
                # Pool tap products; emitted after weights(ci+1) so the
                # channel sums clear Pool's queue first
                nc.gpsimd.tensor_scalar(
                    dummP[:], s_e[ci][0][0:1, 0:1, 0:1], 0.0, None, Alu.add
                )
                nc.gpsimd.tensor_scalar(
                    dummP[:], s_e[ci][2][0:1, 0:1, 0:1], 0.0, None, Alu.add
                )
                ps = psplit(ci)
                pt = pool.tile([ROWS, CS, MAXW], bf16, tag="prod1")
                pt7 = pool.tile([ROWS, CS, MAXW], bf16, tag="prod7")
                for a, b in segs_of(ci):
                    nc.gpsimd.tensor_tensor(
                        pt[:, :, a:b], s_e[ci][0][:, :, 1 + a : 1 + b],
                        wk_view(ci, 1, a, b), Alu.mult,
                    )
                    # Pool's share of the k=7 product
                    pa, pb = a, min(b, ps)
                    if pa < pb:
                        nc.gpsimd.tensor_tensor(
                            pt7[:, :, pa:pb], s_e[ci][2][:, :, 1 + pa : 1 + pb],
                            wk_view(ci, 7, pa, pb), Alu.mult,
                        )
                pp1_all[ci] = pt
                pp7_all[ci] = pt7

            def mac(ci):
                w = CHUNKS[ci]
                ps = psplit(ci)
                wk = wk_all[ci]
                absorb_src_slab(ci)
                nc.vector.tensor_scalar(
                    dummV[:], wk[0:1, 0:1, 0:1], 0.0, None, Alu.add
                )
                prods = {1: pp1_all[ci], 7: pp7_all[ci]}
                # DVE products, ordered by slab arrival (dy 0, 1, 2);
                # k=7's tail columns complement Pool's share
                for k in (0, 2, 3, 5, 6, 8):
                    dy, dx = k // 3, k % 3
                    pt = pool.tile([ROWS, CS, MAXW], bf16, tag=f"prod{k}")
                    for a, b in segs_of(ci):
                        nc.vector.tensor_tensor(
                            pt[:, :, a:b], s_e[ci][dy][:, :, dx + a : dx + b],
                            wk_view(ci, k, a, b), Alu.mult,
                        )
                    prods[k] = pt
                for a, b in segs_of(ci):
                    da, db = max(a, ps), b
                    if da < db:
                        nc.vector.tensor_tensor(
                            prods[7][:, :, da:db],
                            s_e[ci][2][:, :, 1 + da : 1 + db],
                            wk_view(ci, 7, da, db), Alu.mult,
                        )

                # PE part 1: den = sum_k w1_k * wk_k in a 1-bank PSUM tile
                denp = ppool.tile([ROWS, MAXW], fp32, tag="denp")
                dks = (0, 1, 2, 3, 6, 7, 8, 5)
                for a, b in segs_of(ci):
                    for i, k in enumerate(dks):
                        s = SLOT_OF[3 if k == 5 else k]
                        off = (1 if k == 5 else 0) + a
                        nc.tensor.matmul(
                            denp[:, a:b], idw(SIDX[k]),
                            wk[:, s, off : off + (b - a)],
                            start=(i == 0), stop=(i == len(dks) - 1),
                        )
                # den -> SBUF with the center tap's +1 folded into the bias
                dens = pool.tile([ROWS, MAXW], fp32, tag=f"dens{ci}")
                nc.scalar.activation(dens[:, 0:w], denp[:, 0:w], Act.Copy, bias=1.0)
                dens_all[ci] = dens

                # PE part 2: num = center + sum of tap products, in two
                # half-width PSUM tiles; narrow fillers keep DVFS ramped
                half = w // 2
                numps = []
                for h in range(2):
                    np_h = ppool.tile([ROWS, CS, MAXW // 2], fp32, tag=f"nump{h}")
                    numps.append(np_h)
                fill = ppool.tile([ROWS, MAXW], fp32, tag="fill")

                def filler(n):
                    for _ in range(n):
                        nc.tensor.matmul(
                            fill[:], idw(0), s_e[ci][0][:, 0:1, 0:MAXW],
                            start=True, stop=True,
                        )

                acc = [(0, s_e[ci][1][:, :, 1 : 1 + w])]
                acc += [(SIDX[k], prods[k][:, :, 0:w]) for k in (0, 2, 3, 5, 6, 8, 7, 1)]
                n_acc = len(acc)
                for t, (sx, ap) in enumerate(acc):
                    for h in range(2):
                        for a, b in WIN2:
                            nc.tensor.matmul(
                                numps[h][:, a:b, 0:half],
                                idw(sx),
                                ap[:, a:b, h * half : (h + 1) * half],
                                start=(t == 0), stop=(t == n_acc - 1),
                            )
                    if t < n_acc - 1:
                        filler(1)

                # Act: PSUM -> SBUF (bf16) per-half copies of num
                numb = pool.tile([ROWS, CS, MAXW], bf16, tag="numb", bufs=2)
                for h in range(2):
                    nc.scalar.copy(
                        numb[:, :, h * half : (h + 1) * half],
                        numps[h][:, :, 0:half],
                    )

                def finalize():
                    dens = dens_all[ci]
                    rd = pool.tile([ROWS, MAXW], fp32, tag=f"rd{ci}")
                    nc.vector.reciprocal(rd[:, 0:w], dens[:, 0:w])
                    rdb = pool.tile([ROWS, MAXW], bf16, tag=f"rdb{ci}")
                    nc.vector.tensor_scalar(rdb[:, 0:w], rd[:, 0:w], 0.0, None, Alu.add)
                    last = ci == N_CHUNKS - 1
                    outt = pool.tile(
                        [ROWS, CS, CHUNKS[-1] if last else MAXW],
                        bf16, tag="outtL" if last else "outt",
                        bufs=1 if last else 2,
                    )
                    for a, b in [(0, w)]:
                        rdb_b = rdb[:, a:b].rearrange(
                            "p (x w) -> p x w", x=1
                        ).broadcast_to([ROWS, CS, b - a])
                        nc.vector.tensor_tensor(
                            outt[:, :, a:b], numb[:, :, a:b], rdb_b, Alu.mult
                        )
                        if last or w == MAXW:
                            nc.sync.dma_start(out_d[ci][:], outt[:])
                        else:
                            # pad-width DMA: cols >= w are junk, host slices
                            nc.sync.dma_start(out_d[ci][:], outt[:])

                    return None

                return finalize

            pending_finalize = None
            weights(0)
            for ci in range(N_CHUNKS):
                if ci + 1 < N_CHUNKS:
                    weights(ci + 1)
                pp1(ci)
                fin = mac(ci)
                if pending_finalize is not None:
                    pending_finalize()
                pending_finalize = fin
            pending_finalize()
    nc.compile()
    return nc


def _get_nc():
    if "nc" not in _CACHE:
        _CACHE["nc"] = _build()
    return _CACHE["nc"]


def _shard_inputs(src, im):
    srcp = np.pad(src, ((0, 0), (1, 1), (1, 1), (0, 0)), mode="reflect")
    imp = np.pad(im, ((0, 0), (1, 1), (1, 1), (0, 0)), mode="reflect")
    # channel-major: [B, Hp, C, Wp], bf16; pad junk cols so slab/im slices
    # below stay in range
    srcp = np.transpose(srcp, (0, 1, 3, 2)).astype(BF16)
    srcp = np.pad(srcp, ((0, 0), (0, 0), (0, 0), (0, 2)))
    imp = np.transpose(imp, (0, 1, 3, 2)).astype(BF16)
    imp = np.pad(imp, ((0, 0), (0, 0), (0, 0), (0, OFFS[-1] + IMTW - WP)))
    ident = np.stack(
        [(np.eye(ROWS) * math.exp(-0.5 * s)).astype(BF16) for s in range(3)]
    )
    in_maps = []
    for core in range(N_CORES):
        b, r0 = core // 4, (core % 4) * ROWS
        sl = srcp[b, r0 : r0 + ROWS + 2]  # [130, 21, 516]
        imc = imp[b, r0 : r0 + ROWS + 2]  # [130, 3, >=516]
        m = {"ident": ident}
        for ci, w in enumerate(CHUNKS):
            off = OFFS[ci]
            sw = SLABW if w + 2 > SLABW // 2 else w + 2
            m[f"src{ci}"] = np.ascontiguousarray(sl[:, :, off : off + sw])
            m[f"img{ci}"] = np.ascontiguousarray(
                np.stack(
                    [imc[dy : dy + ROWS, :, off : off + IMTW] for dy in range(3)],
                    axis=1,
                )
            )
        in_maps.append(m)
    return in_maps


def kernel(src, im, _trace=False, _tmpdir=None):
    from concourse import bass_utils

    src = np.asarray(src, dtype=np.float32)
    im = np.asarray(im, dtype=np.float32)
    nc = _get_nc()
    in_maps = _shard_inputs(src, im)
    res = bass_utils.run_bass_kernel_spmd(
        nc, in_maps, core_ids=list(range(N_CORES)), trace=_trace, tmpdir=_tmpdir
    )
    out = np.empty((B, H, W, CS), dtype=np.float32)
    for core in range(N_CORES):
        b, r0 = core // 4, (core % 4) * ROWS
        r = res.results[core]
        for ci, w in enumerate(CHUNKS):
            off = OFFS[ci]
            o = r[f"out{ci}"][:, :, 0:w]  # [128, 21, w]
            out[b, r0 : r0 + ROWS, off : off + w, :] = np.transpose(
                o, (0, 2, 1)
            ).astype(np.float32)
    _CACHE["last_results"] = res
    return out


# revision 27
# speedup vs baseline: 1.2273x; 1.1387x over previous
"""Joint bilateral filter (3x3, reflect pad) on 8 trn2 cores.

Sharding: 1024 output rows (2 batches x 512 H) split as 8 x 128 rows.
Host pre-pads H and W with reflect (radius 1), so each core gets a
halo-inclusive channel-major shard and computes its [128, C, 512]
output slab with no boundary handling on device.

Device layout: partition p = output row p of the shard. Free dim is
channel-major [C, W] so the per-pixel bilateral weight (one per W pos)
broadcasts across channels via a stride-0 AP, and dx shifts are free-dim
offsets. dy shifts are handled by loading 3 row-shifted copies of the
inputs (dy = 0,1,2 -> padded rows [dy, dy+128)).

Key structure, per column chunk:
- The 3x3 spatial kernel is folded into PE's weight matrices: the host
  sends three scaled identities w1*I (w1 = exp(-s/2), s = squared tap
  distance in {0,1,2}) and each tap's PSUM accumulate uses the identity
  matching its spatial weight. The guide weights wk are therefore raw
  exp(-8*||guide diff||^2): one batched square, two batched channel-sum
  adds, and ONE batched exp with shared scale for all 7 fields.
- den = 1 + sum_k w1_k*wk_k is ALSO accumulated on PE: eight 128-free
  matmuls (w1*I @ wk_slot) into a 1-bank PSUM tile, +1 via an Act
  Copy-with-bias into SBUF. Pool's only remaining weight work is the
  two batched channel-sum adds.
- DVE:  guide-difference subs (one op per dy group covering its three
  dx shifts via a hand-built overlapping 3-dim AP), ~6.4 of the 8
  non-center tap products in bf16 2x packed mode, reciprocal of den,
  and the final num*(1/den).
- Pool: the k=1 tap product and most of the k=7 product (the column
  split balances DVE vs Pool), plus the channel sums. weights(ci+1)
  is emitted before the Pool products of mac(ci), so the channel sums
  sit early in Pool's queue and the next chunk's exp is never gated
  on Pool's slow products.
- PE:   num = center + sum of 8 tap products via identity-weight
  matmuls into two half-width PSUM tiles (the adds cost the
  otherwise-idle tensor engine ~1.1us per tap instead of 1.5us of
  DVE, and fp32 PSUM accumulation improves accuracy). A narrow filler
  matmul between taps keeps the PE's DVFS ramped through prod waits
  (2.4GHz needs 3us of gap-free execution; any bubble resets it).
- tap symmetry: w5[p,x] = w3[p,x+1] exactly, so tap 5's weight is an
  offset view into k=3's slot of the batched weight tile.
- chunk widths are [112, 128, 128, 112, 32]: a short first chunk
  (split further into two column halves all the way through
  weights+mac) shortens the serial startup of the weight pipeline,
  and a tiny last chunk shrinks the drain tail. All SBUF tiles are
  allocated at the 128-col maximum (same-tag tiles must keep one
  shape); narrower chunks just use a prefix, and their DMAs write a
  sliced prefix of the tile.
- src arrives as per-chunk row-shifted slabs, one contiguous run per
  partition; the guide arrives as ONE merged per-chunk DMA and the
  identities as one merged DMA - HWDGE grants cost 625ns each, so
  fewer DMA dispatches start the pipeline sooner. The output DRAM is
  per-chunk so each out-DMA descriptor is one contiguous multi-KB run
  per partition (runs under 512B pay 2x in the DMA engines).
- each chunk's finalize (reciprocal etc) is emitted one chunk late so
  the in-order DVE stream never stalls on den/num completion.
- center tap weight is exactly 1: PE accumulates src directly, +1 for
  den via the Act bias.
"""

import sys

sys.path.insert(0, "/opt/trn_rl_repo")

import math

import ml_dtypes
import numpy as np

BF16 = ml_dtypes.bfloat16

B, H, W = 2, 512, 512
CS, CI = 21, 3
N_CORES = 8
ROWS = (B * H) // N_CORES  # 128 output rows per core
WP = W + 2  # padded width

CHUNKS = [112, 128, 128, 112, 32]
OFFS = [0, 112, 240, 368, 480]
N_CHUNKS = len(CHUNKS)
MAXW = 128
SLABW = MAXW + 2  # slab tile width
IMTW = MAXW + 4  # guide tile width

SIGMA_BILATERAL = 0.25
INV2SIG2 = 1.0 / (2.0 * SIGMA_BILATERAL**2)  # 8.0

# weight-field slots in the batched D/WR/WK tiles:
#   slot 0..2 = taps (dy=0, dx=0..2) = k0,k1,k2
#   slot 3..5 = taps (dy=2, dx=0..2) = k6,k7,k8
#   slot 6    = tap  (dy=1, dx=0)    = k3   (k5 = slot 6, one col right)
SLOT_OF = {0: 0, 1: 1, 2: 2, 6: 3, 7: 4, 8: 5, 3: 6}
# spatial-kernel index per tap: (dx-1)^2 + (dy-1)^2 in {0,1,2}
SIDX = {k: ((k % 3) - 1) ** 2 + ((k // 3) - 1) ** 2 for k in range(9)}

_CACHE = {}


def _ap_with(ap, dims):
    """Copy `ap` and replace its free dims (keeps partition dim + offset)."""
    import bass_rust

    c = ap.copy()
    part = list(c.ap)[0]
    c.ap = bass_rust.VecI64Pair([list(part)] + [list(d) for d in dims])
    return c


def _build():
    from concourse.bacc import Bacc
    from concourse.tile import TileContext
    import concourse.mybir as mybir

    fp32 = mybir.dt.float32
    bf16 = mybir.dt.bfloat16
    Alu = mybir.AluOpType
    Act = mybir.ActivationFunctionType

    nc = Bacc("TRN2", target_bir_lowering=False, debug=False, num_devices=N_CORES)
    se_d, im_d, out_d = [], [], []
    for ci, w in enumerate(CHUNKS):
        # slabs/guides of near-max chunks are padded to the tile width so
        # the load stays one contiguous >=512B run per partition; only the
        # tiny tail chunk uses a sliced (min-time-descriptor) load
        sw = SLABW if w + 2 > SLABW // 2 else w + 2
        se_d.append(
            nc.dram_tensor(f"src{ci}", [ROWS + 2, CS, sw], bf16, kind="ExternalInput")
        )
        im_d.append(
            nc.dram_tensor(f"img{ci}", [ROWS, 3, CI, IMTW], bf16, kind="ExternalInput")
        )
        ow = MAXW if ci < N_CHUNKS - 1 else w
        out_d.append(nc.dram_tensor(f"out{ci}", [ROWS, CS, ow], bf16, kind="ExternalOutput"))
    # three spatially-scaled identities: exp(-s/2) * I for s = 0, 1, 2
    id_d = nc.dram_tensor("ident", [3, ROWS, ROWS], bf16, kind="ExternalInput")

    # num PSUM half-windows: channel ranges of <= 8 channels
    # (8*64 = 512 fp32, one 2KB bank per window)
    WIN2 = [(c0, min(CS, c0 + 8)) for c0 in range(0, CS, 8)]

    with TileContext(nc) as tc:
        with tc.tile_pool(name="p", bufs=1) as pool, tc.tile_pool(
            name="ps", bufs=1, space="PSUM"
        ) as ppool:
            # ---- input DMAs, interleaved for a fast chunk-0 start -------
            imt = [None] * N_CHUNKS
            s_e = [[None] * 3 for _ in range(N_CHUNKS)]

            def issue_im(ci):
                it = pool.tile([ROWS, 3, CI, IMTW], bf16, tag="imt", bufs=2)
                nc.sync.dma_start(
                    it[:].rearrange("p a c w -> p (a c w)"),
                    im_d[ci][:].rearrange("p a c w -> p (a c w)"),
                )
                imt[ci] = it

            def issue_slab(ci, dys):
                w = CHUNKS[ci]
                sw = SLABW if w + 2 > SLABW // 2 else w + 2
                for dy in dys:
                    se = pool.tile([ROWS, CS, SLABW], bf16, tag=f"se{dy}", bufs=2)
                    src = se_d[ci][dy : dy + ROWS]
                    if sw == SLABW:
                        nc.sync.dma_start(
                            se[:].rearrange("p c w -> p (c w)"),
                            src.rearrange("p c w -> p (c w)"),
                        )
                    else:
                        nc.sync.dma_start(se[:, :, 0:sw], src)
                    s_e[ci][dy] = se

            issue_im(0)
            issue_slab(0, (0,))
            ident = pool.tile([ROWS, 3, ROWS], bf16, tag="ident")
            nc.sync.dma_start(ident[:], id_d[:].transpose([1, 0, 2]))
            issue_slab(0, (1, 2))
            for ci in range(1, N_CHUNKS):
                issue_im(ci)
                issue_slab(ci, (0, 1, 2))

            def idw(s):
                return ident[:, s]

            # Each consuming engine observes every slab DMA once (tiny
            # absorber ops) so real consumers don't pile up sync waits.
            dummV = pool.tile([1, 1, 1], bf16, tag="dummV")
            dummP = pool.tile([1, 1, 1], bf16, tag="dummP")

            def absorb_src_slab(ci):
                for t in s_e[ci]:
                    nc.vector.tensor_scalar(
                        dummV[:], t[0:1, 0:1, 0:1], 0.0, None, Alu.add
                    )

            # --- software-pipelined chunk loop ---------------------------
            wk_all = [None] * N_CHUNKS
            dens_all = [None] * N_CHUNKS
            pp1_all = [None] * N_CHUNKS
            pp7_all = [None] * N_CHUNKS

            def segs_of(ci):
                w = CHUNKS[ci]
                return [(0, w // 2), (w // 2, w)] if ci == 0 else [(0, w)]

            def psplit(ci):
                w = CHUNKS[ci]
                return (w * 5) // 8 if w >= 64 else 0

            def wk_view(ci, k, a, b):
                wk = wk_all[ci]
                s = SLOT_OF[3 if k == 5 else k]
                off = (1 if k == 5 else 0) + a
                return (
                    wk[:, s : s + 1, off : off + (b - a)]
                    .broadcast_to([ROWS, CS, b - a])
                )

            def weights(ci):
                w = CHUNKS[ci]
                it = imt[ci]
                d = pool.tile([ROWS, 7, CI, SLABW], bf16, tag="d", bufs=2)
                d2 = pool.tile([ROWS, 7, CI, SLABW], fp32, tag="d2", bufs=2)
                wr = pool.tile([ROWS, 7, SLABW], fp32, tag="wr", bufs=2)
                wk = pool.tile([ROWS, 7, SLABW], bf16, tag="wk", bufs=2)
                dcw = CI * SLABW
                fsegs = (
                    [(0, w // 2 + 2), (w // 2 + 2, w + 2)] if ci == 0 else [(0, w + 2)]
                )
                for a, b in fsegs:
                    n = b - a
                    # the three dx-shifted fields of each row-shifted dy
                    # group in ONE sub via an overlapping (dx, c, x) AP
                    # (engine APs allow at most 3 free dims: one op per dy)
                    for g, dy in enumerate((0, 2)):
                        mn3 = _ap_with(it[:, dy, :, a:], [[1, 3], [IMTW, 3], [1, n]])
                        ct3 = _ap_with(
                            it[:, 1, :, a + 1 :], [[0, 3], [IMTW, 3], [1, n]]
                        )
                        o3 = _ap_with(d[:, 3 * g, :, a:], [[dcw, 3], [SLABW, 3], [1, n]])
                        nc.vector.tensor_tensor(o3, mn3, ct3, Alu.subtract)
                    # k=3 field (dy=1, dx=0)
                    nc.vector.tensor_tensor(
                        d[:, 6, :, a:b], it[:, 1, :, a:b],
                        it[:, 1, :, a + 1 : b + 1], Alu.subtract,
                    )
                    # one batched square of all 7 fields
                    nc.scalar.square(d2[:, :, :, a:b], d[:, :, :, a:b])
                    # channel sums on Pool, batched across the 7 fields
                    nc.gpsimd.tensor_tensor(
                        wr[:, :, a:b], d2[:, :, 0, a:b], d2[:, :, 1, a:b], Alu.add
                    )
                    nc.gpsimd.tensor_tensor(
                        wr[:, :, a:b], d2[:, :, 2, a:b], wr[:, :, a:b], Alu.add
                    )
                    # one batched exp (raw guide weights; spatial factors
                    # live in PE's scaled identities)
                    nc.scalar.activation(
                        wk[:, :, a:b], wr[:, :, a:b], Act.Exp,
                        bias=0.0, scale=-INV2SIG2,
                    )
                wk_all[ci] = wk

            def pp1(ci):
                # Pool tap products; emitted after weights(ci+1) so the
                # channel sums clear Pool's queue first
                nc.gpsimd.tensor_scalar(
                    dummP[:], s_e[ci][0][0:1, 0:1, 0:1], 0.0, None, Alu.add
                )
                nc.gpsimd.tensor_scalar(
                    dummP[:], s_e[ci][2][0:1, 0:1, 0:1], 0.0, None, Alu.add
                )
                ps = psplit(ci)
                pt = pool.tile([ROWS, CS, MAXW], bf16, tag="prod1", bufs=2)
                pt7 = pool.tile([ROWS, CS, MAXW], bf16, tag="prod7", bufs=2)
                for a, b in segs_of(ci):
                    nc.gpsimd.tensor_tensor(
                        pt[:, :, a:b], s_e[ci][0][:, :, 1 + a : 1 + b],
                        wk_view(ci, 1, a, b), Alu.mult,
                    )
                    # Pool's share of the k=7 product
                    pa, pb = a, min(b, ps)
                    if pa < pb:
                        nc.gpsimd.tensor_tensor(
                            pt7[:, :, pa:pb], s_e[ci][2][:, :, 1 + pa : 1 + pb],
                            wk_view(ci, 7, pa, pb), Alu.mult,
                        )
                pp1_all[ci] = pt
                pp7_all[ci] = pt7

            def mac(ci):
                w = CHUNKS[ci]
                ps = psplit(ci)
                wk = wk_all[ci]
                absorb_src_slab(ci)
                nc.vector.tensor_scalar(
                    dummV[:], wk[0:1, 0:1, 0:1], 0.0, None, Alu.add
                )
                prods = {1: pp1_all[ci], 7: pp7_all[ci]}
                # DVE products, ordered by slab arrival (dy 0, 1, 2);
                # k=7's tail columns complement Pool's share
                for k in (0, 2, 3, 5, 6, 8):
                    dy, dx = k // 3, k % 3
                    pt = pool.tile([ROWS, CS, MAXW], bf16, tag=f"prod{k}", bufs=2)
                    for a, b in segs_of(ci):
                        nc.vector.tensor_tensor(
                            pt[:, :, a:b], s_e[ci][dy][:, :, dx + a : dx + b],
                            wk_view(ci, k, a, b), Alu.mult,
                        )
                    prods[k] = pt
                for a, b in segs_of(ci):
                    da, db = max(a, ps), b
                    if da < db:
                        nc.vector.tensor_tensor(
                            prods[7][:, :, da:db],
                            s_e[ci][2][:, :, 1 + da : 1 + db],
                            wk_view(ci, 7, da, db), Alu.mult,
                        )

                # PE part 1: den = sum_k w1_k * wk_k in a 1-bank PSUM tile
                denp = ppool.tile([ROWS, MAXW], fp32, tag="denp")
                dks = (0, 1, 2, 3, 6, 7, 8, 5)
                for a, b in segs_of(ci):
                    for i, k in enumerate(dks):
                        s = SLOT_OF[3 if k == 5 else k]
                        off = (1 if k == 5 else 0) + a
                        nc.tensor.matmul(
                            denp[:, a:b], idw(SIDX[k]),
                            wk[:, s, off : off + (b - a)],
                            start=(i == 0), stop=(i == len(dks) - 1),
                        )
                # den -> SBUF with the center tap's +1 folded into the bias
                dens = pool.tile([ROWS, MAXW], fp32, tag=f"dens{ci}")
                nc.scalar.activation(dens[:, 0:w], denp[:, 0:w], Act.Copy, bias=1.0)
                dens_all[ci] = dens

                # PE part 2: num = center + sum of tap products, in two
                # half-width PSUM tiles; narrow fillers keep DVFS ramped
                half = w // 2
                numps = []
                for h in range(2):
                    np_h = ppool.tile([ROWS, CS, MAXW // 2], fp32, tag=f"nump{h}")
                    numps.append(np_h)
                fill = ppool.tile([ROWS, MAXW], fp32, tag="fill")

                def filler(n):
                    for _ in range(n):
                        nc.tensor.matmul(
                            fill[:], idw(0), s_e[ci][0][:, 0:1, 0:MAXW],
                            start=True, stop=True,
                        )

                acc = [(0, s_e[ci][1][:, :, 1 : 1 + w])]
                acc += [(SIDX[k], prods[k][:, :, 0:w]) for k in (0, 2, 3, 5, 6, 8, 7, 1)]
                n_acc = len(acc)
                for t, (sx, ap) in enumerate(acc):
                    for h in range(2):
                        for a, b in WIN2:
                            nc.tensor.matmul(
                                numps[h][:, a:b, 0:half],
                                idw(sx),
                                ap[:, a:b, h * half : (h + 1) * half],
                                start=(t == 0), stop=(t == n_acc - 1),
                            )
                    if t < n_acc - 1:
                        filler(1)

                # Act: PSUM -> SBUF (bf16) per-half copies of num
                numb = pool.tile([ROWS, CS, MAXW], bf16, tag="numb", bufs=2)
                for h in range(2):
                    nc.scalar.copy(
                        numb[:, :, h * half : (h + 1) * half],
                        numps[h][:, :, 0:half],
                    )

                def finalize():
                    dens = dens_all[ci]
                    rd = pool.tile([ROWS, MAXW], fp32, tag=f"rd{ci}")
                    nc.vector.reciprocal(rd[:, 0:w], dens[:, 0:w])
                    rdb = pool.tile([ROWS, MAXW], bf16, tag=f"rdb{ci}")
                    nc.vector.tensor_scalar(rdb[:, 0:w], rd[:, 0:w], 0.0, None, Alu.add)
                    last = ci == N_CHUNKS - 1
                    outt = pool.tile(
                        [ROWS, CS, CHUNKS[-1] if last else MAXW],
                        bf16, tag="outtL" if last else "outt",
                        bufs=1 if last else 2,
                    )
                    for a, b in [(0, w)]:
                        rdb_b = rdb[:, a:b].rearrange(
                            "p (x w) -> p x w", x=1
                        ).broadcast_to([ROWS, CS, b - a])
                        nc.vector.tensor_tensor(
                            outt[:, :, a:b], numb[:, :, a:b], rdb_b, Alu.mult
                        )
                        if last or w == MAXW:
                            nc.sync.dma_start(out_d[ci][:], outt[:])
                        else:
                            # pad-width DMA: cols >= w are junk, host slices
                            nc.sync.dma_start(out_d[ci][:], outt[:])

                    return None

                return finalize

            pending_finalize = None
            weights(0)
            for ci in range(N_CHUNKS):
                if ci + 1 < N_CHUNKS:
                    weights(ci + 1)
                pp1(ci)
                fin = mac(ci)
                if pending_finalize is not None:
                    pending_finalize()
                pending_finalize = fin
            pending_finalize()
    nc.compile()
    return nc


def _get_nc():
    if "nc" not in _CACHE:
        _CACHE["nc"] = _build()
    return _CACHE["nc"]


def _shard_inputs(src, im):
    srcp = np.pad(src, ((0, 0), (1, 1), (1, 1), (0, 0)), mode="reflect")
    imp = np.pad(im, ((0, 0), (1, 1), (1, 1), (0, 0)), mode="reflect")
    # channel-major: [B, Hp, C, Wp], bf16; pad junk cols so slab/im slices
    # below stay in range
    srcp = np.transpose(srcp, (0, 1, 3, 2)).astype(BF16)
    srcp = np.pad(srcp, ((0, 0), (0, 0), (0, 0), (0, 2)))
    imp = np.transpose(imp, (0, 1, 3, 2)).astype(BF16)
    imp = np.pad(imp, ((0, 0), (0, 0), (0, 0), (0, OFFS[-1] + IMTW - WP)))
    ident = np.stack(
        [(np.eye(ROWS) * math.exp(-0.5 * s)).astype(BF16) for s in range(3)]
    )
    in_maps = []
    for core in range(N_CORES):
        b, r0 = core // 4, (core % 4) * ROWS
        sl = srcp[b, r0 : r0 + ROWS + 2]  # [130, 21, 516]
        imc = imp[b, r0 : r0 + ROWS + 2]  # [130, 3, >=516]
        m = {"ident": ident}
        for ci, w in enumerate(CHUNKS):
            off = OFFS[ci]
            sw = SLABW if w + 2 > SLABW // 2 else w + 2
            m[f"src{ci}"] = np.ascontiguousarray(sl[:, :, off : off + sw])
            m[f"img{ci}"] = np.ascontiguousarray(
                np.stack(
                    [imc[dy : dy + ROWS, :, off : off + IMTW] for dy in range(3)],
                    axis=1,
                )
            )
        in_maps.append(m)
    return in_maps


def kernel(src, im, _trace=False, _tmpdir=None):
    from concourse import bass_utils

    src = np.asarray(src, dtype=np.float32)
    im = np.asarray(im, dtype=np.float32)
    nc = _get_nc()
    in_maps = _shard_inputs(src, im)
    res = bass_utils.run_bass_kernel_spmd(
        nc, in_maps, core_ids=list(range(N_CORES)), trace=_trace, tmpdir=_tmpdir
    )
    out = np.empty((B, H, W, CS), dtype=np.float32)
    for core in range(N_CORES):
        b, r0 = core // 4, (core % 4) * ROWS
        r = res.results[core]
        for ci, w in enumerate(CHUNKS):
            off = OFFS[ci]
            o = r[f"out{ci}"][:, :, 0:w]  # [128, 21, w]
            out[b, r0 : r0 + ROWS, off : off + w, :] = np.transpose(
                o, (0, 2, 1)
            ).astype(np.float32)
    _CACHE["last_results"] = res
    return out
